# revision 12
# baseline (speedup 1.0000x reference)
import os
import sys

sys.path.insert(0, "/opt/trn_rl_repo")

import numpy as np

import concourse.bass as bass
import concourse.tile as tile
from concourse import mybir
from concourse.alu_op_type import AluOpType
from concourse.bass_utils import run_bass_kernel_spmd
from concourse.masks import make_identity

import orjson


def _split_multiwait(d: dict) -> dict:
    cnt = 0
    for fn in d.get("functions", []):
        for blk in fn.get("blocks", []):
            insts = blk.get("instructions", [])
            if not any(
                len((i.get("sync_info") or {}).get("on_wait") or []) > 1
                for i in insts
            ):
                continue
            new = []
            for ins in insts:
                si = ins.get("sync_info") or {}
                waits = si.get("on_wait") or []
                if len(waits) > 1:
                    for w in waits[:-1]:
                        cnt += 1
                        nop = {
                            "engine": ins["engine"],
                            "ins": [],
                            "outs": [],
                            "name": f"I-waitsplit-{cnt}",
                            "opcode": "NoOp",
                            "sync_info": {"on_update": [], "on_wait": [w]},
                            "text_hint": "wait_split",
                        }
                        if "debug" in ins:
                            nop["debug"] = ins["debug"]
                        new.append(nop)
                    si["on_wait"] = [waits[-1]]
                new.append(ins)
            blk["instructions"] = new
    return d


_orig_to_json_bytes = bass.Bass.to_json_bytes


def _patched_to_json_bytes(self) -> bytes:
    return orjson.dumps(_split_multiwait(orjson.loads(_orig_to_json_bytes(self))))


bass.Bass.to_json_bytes = _patched_to_json_bytes

N_CORES = 8
B, C_IN, H, WW = 16, 256, 2000, 9
C_OUT = 128
B_LOC = B // N_CORES
G, D = 4, 32
SCALE = 1.0 / np.sqrt(32.0)
MASK_A = float(np.sqrt(245.25))
MASK_S = float(np.sqrt(200.0))
HC = 14
PC = HC * WW
SUPER_H = 56
F16 = mybir.dt.float16
F32 = mybir.dt.float32
F32R = mybir.dt.float32r
EXP = mybir.ActivationFunctionType.Exp


def _superchunks():
    out = []
    h0 = 0
    while h0 + SUPER_H <= H:
        out.append((h0, SUPER_H))
        h0 += SUPER_H
    if h0 < H:
        out.append((h0, H - h0))
    return out


def _chunks(hcnt):
    out = []
    c0 = 0
    while c0 < hcnt:
        out.append((c0, min(HC, hcnt - c0)))
        c0 += HC
    return out


def _host_consts():
    maskL = np.zeros((128, 128), dtype=np.float16)
    maskR = np.zeros((128, 512), dtype=np.float16)
    for g in range(G):
        r0 = 32 * g
        maskL[r0, :PC] = MASK_A
        for ci in range(4):
            cb = 126 * ci
            maskR[r0, cb:cb + PC] = -MASK_A
            for hb in range(HC):
                maskL[r0 + 1 + hb, hb * WW:(hb + 1) * WW] = MASK_S
                maskR[r0 + 1 + hb, cb + hb * WW:cb + (hb + 1) * WW] = MASK_S
    return maskL, maskR


def _build_kernel():
    nc = bass.Bass("TRN2")
    X = nc.declare_dram_parameter("X", [B_LOC, C_IN, H, WW], F32R, isOutput=False)
    WT = nc.declare_dram_parameter("WT", [C_IN, C_OUT], F32R, isOutput=False)
    BIAS = nc.declare_dram_parameter("BIAS", [C_OUT, 1], F32, isOutput=False)
    ML = nc.declare_dram_parameter("ML", [128, 128], F16, isOutput=False)
    MR = nc.declare_dram_parameter("MR", [128, 512], F16, isOutput=False)
    OUT = nc.declare_dram_parameter("OUT", [B_LOC, C_OUT, H, WW], F32, isOutput=True)

    from contextlib import ExitStack
    with tile.TileContext(nc) as tc, ExitStack() as ctx:
        sb1 = ctx.enter_context(tc.tile_pool(name="sb1", bufs=1))
        sbx = ctx.enter_context(tc.tile_pool(name="sbx", bufs=2))
        sbf = ctx.enter_context(tc.tile_pool(name="sbf", bufs=2))
        sbo = ctx.enter_context(tc.tile_pool(name="sbo", bufs=2))
        sba = ctx.enter_context(tc.tile_pool(name="sba", bufs=2))
        psf = ctx.enter_context(tc.tile_pool(name="psf", bufs=1, space="PSUM"))
        psx = ctx.enter_context(tc.tile_pool(name="psx", bufs=1, space="PSUM"))
        pss = ctx.enter_context(tc.tile_pool(name="pss", bufs=1, space="PSUM"))
        psa = ctx.enter_context(tc.tile_pool(name="psa", bufs=1, space="PSUM"))

        wt0 = sb1.tile([128, C_OUT], F32R, name="wt0")
        wt1 = sb1.tile([128, C_OUT], F32R, name="wt1")
        nc.gpsimd.dma_start(out=wt0, in_=WT[0:128, :])
        nc.gpsimd.dma_start(out=wt1, in_=WT[128:256, :])
        bias = sb1.tile([C_OUT, 1], F32, name="bias")
        nc.gpsimd.dma_start(out=bias, in_=BIAS[:])
        ml = sb1.tile([128, 128], F16, name="ml")
        mr = sb1.tile([128, 512], F16, name="mr")
        nc.gpsimd.dma_start(out=ml, in_=ML[:])
        nc.gpsimd.dma_start(out=mr, in_=MR[:])
        ident = sb1.tile([128, 128], F16, name="ident")
        make_identity(nc, ident)
        ones32 = sb1.tile([128, 32], F16, name="ones32")
        nc.vector.memset(ones32, 1.0)

        for (h0, hcnt) in _superchunks():
            n = hcnt * WW
            nch = _chunks(hcnt)
            nlast = len(nch) - 1
            ncols = 128 * len(nch)
            xx = sbx.tile([128, 4 * 512], F32R, tag="xx")
            xxv = xx.rearrange("c (b k m) -> c b k m", b=2, k=2)
            nc.sync.dma_start(
                out=xxv[:, :, :, :n],
                in_=X[:, :, h0:h0 + hcnt, :].rearrange(
                    "b (k c) h w -> c b k (h w)", k=2
                ),
            )
            o = sbo.tile([C_OUT, 2 * 512], F32, tag="o")
            for bi in range(B_LOC):
                xb = 1024 * bi
                pfeat = psf.tile([C_OUT, 512], F32, tag="pfeat")
                nc.tensor.matmul(pfeat[:, :n], wt0, xx[:, xb:xb + n],
                                 start=True, stop=False)
                nc.tensor.matmul(pfeat[:, :n], wt1, xx[:, xb + 512:xb + 512 + n],
                                 start=False, stop=True)
                feat = sbf.tile([C_OUT, 512], F16, tag="feat")
                nc.vector.tensor_scalar(
                    out=feat[:, :n], in0=pfeat[:, :n],
                    scalar1=bias, scalar2=None, op0=AluOpType.add,
                )
                if n < 512:
                    nc.vector.memset(feat[:, n:], 0.0)

                xT = psx.tile([128, 512], F16, tag="xT")
                for ci, (c0, hc) in enumerate(nch):
                    nc.tensor.transpose(
                        xT[:PC, 128 * ci:128 * ci + 128],
                        feat[:, c0 * WW:c0 * WW + PC], ident,
                    )
                texp = sba.tile([PC, 512], F16, tag="texp")
                nc.scalar.activation(out=texp[:, :ncols], in_=xT[:PC, :ncols],
                                     func=EXP)
                u1 = sba.tile([PC, 512], F16, tag="u1")
                nc.gpsimd.tensor_scalar(out=u1[:, :ncols], in0=texp[:, :ncols],
                                        scalar1=1.0, scalar2=-1.0,
                                        op0=AluOpType.min, op1=AluOpType.add)
                val = sba.tile([PC, 512], F16, tag="val")
                nc.vector.scalar_tensor_tensor(out=val[:, :ncols],
                                               in0=xT[:PC, :ncols],
                                               scalar=0.0, in1=u1[:, :ncols],
                                               op0=AluOpType.max,
                                               op1=AluOpType.add)

                ssc = pss.tile([128, 2048], F32, tag="ssc")
                eall = sba.tile([128, 2048], F16, tag="eall")
                for g in range(G):
                    r0 = 32 * g
                    nc.tensor.matmul(
                        ssc[:, 512 * g:512 * g + 504],
                        ml[r0:r0 + 15, :], mr[r0:r0 + 15, :504],
                        start=True, stop=False, tile_position=(r0, 0),
                    )
                    for ci, (c0, hc) in enumerate(nch):
                        p = hc * WW
                        col = 512 * g + 126 * ci
                        nc.tensor.matmul(
                            ssc[:, col:col + p],
                            feat[r0:r0 + 32, c0 * WW:c0 * WW + 128],
                            feat[r0:r0 + 32, c0 * WW:c0 * WW + p],
                            start=False, stop=(ci == nlast),
                            tile_position=(r0, 0),
                        )
                nc.scalar.activation(
                    out=eall.rearrange("q (g m) -> q g m", m=512)[:, :, :504],
                    in_=ssc.rearrange("q (g m) -> q g m", m=512)[:, :, :504],
                    func=EXP, scale=SCALE,
                )

                av = psa.tile([C_OUT, 512], F32, tag="av")
                rr = psa.tile([C_OUT, 512], F32, tag="rr")
                for ci, (c0, hc) in enumerate(nch):
                    p = hc * WW
                    for g in range(G):
                        r0 = 32 * g
                        esl = eall[:p, 512 * g + 126 * ci:512 * g + 126 * ci + p]
                        nc.tensor.matmul(
                            av[r0:r0 + 32, c0 * WW:c0 * WW + p],
                            val[:p, 128 * ci + r0:128 * ci + r0 + 32], esl,
                            start=True, stop=True, tile_position=(0, r0),
                        )
                        nc.tensor.matmul(
                            rr[r0:r0 + 32, c0 * WW:c0 * WW + p],
                            ones32[:p, :], esl,
                            start=True, stop=True, tile_position=(0, r0),
                        )
                rrs = sba.tile([C_OUT, 512], F32, tag="rrs")
                nc.vector.tensor_scalar(out=rrs[:, :n], in0=rr[:, :n],
                                        scalar1=0.0, scalar2=None,
                                        op0=AluOpType.add)
                rcp = sba.tile([C_OUT, 512], F32, tag="rcp")
                nc.gpsimd.tensor_scalar(
                    out=rcp[:, :n].bitcast(mybir.dt.int32),
                    in0=rrs[:, :n].bitcast(mybir.dt.int32),
                    scalar1=0x7EF311C3, scalar2=-1,
                    op0=AluOpType.subtract, op1=AluOpType.mult)
                nrt_ = sba.tile([C_OUT, 512], F32, tag="nrt")
                nc.gpsimd.tensor_tensor(out=nrt_[:, :n], in0=rrs[:, :n],
                                        in1=rcp[:, :n], op=AluOpType.mult)
                nc.gpsimd.tensor_scalar(out=nrt_[:, :n], in0=nrt_[:, :n],
                                        scalar1=-1.0, scalar2=2.0,
                                        op0=AluOpType.mult, op1=AluOpType.add)
                nc.gpsimd.tensor_tensor(out=rcp[:, :n], in0=rcp[:, :n],
                                        in1=nrt_[:, :n], op=AluOpType.mult)
                nc.vector.scalar_tensor_tensor(
                    out=o[:, 512 * bi:512 * bi + n], in0=av[:, :n],
                    scalar=0.0, in1=rcp[:, :n],
                    op0=AluOpType.add, op1=AluOpType.mult,
                )

            nc.sync.dma_start(
                out=OUT[:, :, h0:h0 + hcnt, :].rearrange("b c h w -> c b (h w)"),
                in_=o.rearrange("c (b m) -> c b m", b=2)[:, :, :n],
            )

    return nc


_CACHED = {}


def _get_nc():
    if "nc" not in _CACHED:
        _CACHED["nc"] = _build_kernel()
    return _CACHED["nc"]


def _make_in_maps(inputs) -> list:
    x = np.asarray(inputs["neighbr_feats"], dtype=np.float32)
    w = np.asarray(inputs["W"], dtype=np.float32)
    b = np.asarray(inputs["b"], dtype=np.float32)

    wt = np.ascontiguousarray(w.T)
    bias = np.ascontiguousarray(b.reshape(C_OUT, 1))
    maskL, maskR = _host_consts()

    in_maps = []
    for core in range(N_CORES):
        xs = np.ascontiguousarray(x[core * B_LOC:(core + 1) * B_LOC])
        in_maps.append({
            "X": xs, "WT": wt, "BIAS": bias, "ML": maskL, "MR": maskR,
        })
    return in_maps


def kernel(**inputs) -> np.ndarray:
    nc = _get_nc()
    in_maps = _make_in_maps(inputs)
    res = run_bass_kernel_spmd(nc, in_maps, core_ids=list(range(N_CORES)))
    out = np.concatenate([r["OUT"] for r in res.results], axis=0)
    return out.astype(np.float32)


if __name__ == "__main__":
    rng = np.random.default_rng(0)
    inputs = {
        "neighbr_feats": rng.standard_normal((B, C_IN, H, WW)).astype(np.float32),
        "W": (rng.standard_normal((C_OUT, C_IN)) * 0.05).astype(np.float32),
        "b": (rng.standard_normal((C_OUT,)) * 0.05).astype(np.float32),
    }
    out = kernel(**inputs)
    print("kernel ran:", out.shape, out.dtype)


# revision 17
# speedup vs baseline: 1.8763x; 1.8763x over previous
import os
import sys

sys.path.insert(0, "/opt/trn_rl_repo")

import numpy as np

import concourse.bass as bass
import concourse.tile as tile
from concourse import mybir
from concourse.alu_op_type import AluOpType
from concourse.bass_utils import run_bass_kernel_spmd
from concourse.masks import make_identity

import orjson


def _split_multiwait(d: dict) -> dict:
    cnt = 0
    for fn in d.get("functions", []):
        for blk in fn.get("blocks", []):
            insts = blk.get("instructions", [])
            if not any(
                len((i.get("sync_info") or {}).get("on_wait") or []) > 1
                for i in insts
            ):
                continue
            new = []
            for ins in insts:
                si = ins.get("sync_info") or {}
                waits = si.get("on_wait") or []
                if len(waits) > 1:
                    for w in waits[:-1]:
                        cnt += 1
                        nop = {
                            "engine": ins["engine"],
                            "ins": [],
                            "outs": [],
                            "name": f"I-waitsplit-{cnt}",
                            "opcode": "NoOp",
                            "sync_info": {"on_update": [], "on_wait": [w]},
                            "text_hint": "wait_split",
                        }
                        if "debug" in ins:
                            nop["debug"] = ins["debug"]
                        new.append(nop)
                    si["on_wait"] = [waits[-1]]
                new.append(ins)
            blk["instructions"] = new
    return d


_orig_to_json_bytes = bass.Bass.to_json_bytes


def _patched_to_json_bytes(self) -> bytes:
    return orjson.dumps(_split_multiwait(orjson.loads(_orig_to_json_bytes(self))))


bass.Bass.to_json_bytes = _patched_to_json_bytes

N_CORES = 8
B, C_IN, H, WW = 16, 256, 2000, 9
C_OUT = 128
B_LOC = B // N_CORES
G, D = 4, 32
SCALE = 1.0 / np.sqrt(32.0)
MASK_A = float(np.sqrt(245.25))
MASK_S = float(np.sqrt(200.0))
HC = 14
PC = HC * WW
SUPER_H = 56
F16 = mybir.dt.float16
F32 = mybir.dt.float32
F32R = mybir.dt.float32r
EXP = mybir.ActivationFunctionType.Exp


def _superchunks():
    out = []
    h0 = 0
    while h0 + SUPER_H <= H:
        out.append((h0, SUPER_H))
        h0 += SUPER_H
    if h0 < H:
        out.append((h0, H - h0))
    return out


def _chunks(hcnt):
    out = []
    c0 = 0
    while c0 < hcnt:
        out.append((c0, min(HC, hcnt - c0)))
        c0 += HC
    return out


def _host_consts():
    maskL = np.zeros((128, 128), dtype=np.float16)
    maskR = np.zeros((128, 512), dtype=np.float16)
    for g in range(G):
        r0 = 32 * g
        maskL[r0, :PC] = MASK_A
        for ci in range(4):
            cb = 126 * ci
            maskR[r0, cb:cb + PC] = -MASK_A
            for hb in range(HC):
                maskL[r0 + 1 + hb, hb * WW:(hb + 1) * WW] = MASK_S
                maskR[r0 + 1 + hb, cb + hb * WW:cb + (hb + 1) * WW] = MASK_S
    return maskL, maskR


def _build_kernel():
    nc = bass.Bass("TRN2")
    X = nc.declare_dram_parameter("X", [B_LOC, C_IN, H, WW], F32R, isOutput=False)
    WT = nc.declare_dram_parameter("WT", [C_IN, C_OUT], F32R, isOutput=False)
    BIAS = nc.declare_dram_parameter("BIAS", [C_OUT, 1], F32, isOutput=False)
    ML = nc.declare_dram_parameter("ML", [128, 128], F16, isOutput=False)
    MR = nc.declare_dram_parameter("MR", [128, 512], F16, isOutput=False)
    OUT = nc.declare_dram_parameter("OUT", [B_LOC, C_OUT, H, WW], F32, isOutput=True)

    from contextlib import ExitStack
    with tile.TileContext(nc) as tc, ExitStack() as ctx:
        sb1 = ctx.enter_context(tc.tile_pool(name="sb1", bufs=1))
        sbx = ctx.enter_context(tc.tile_pool(name="sbx", bufs=2))
        sbf = ctx.enter_context(tc.tile_pool(name="sbf", bufs=2))
        sbo = ctx.enter_context(tc.tile_pool(name="sbo", bufs=2))
        sba = ctx.enter_context(tc.tile_pool(name="sba", bufs=2))
        psf = ctx.enter_context(tc.tile_pool(name="psf", bufs=2, space="PSUM"))
        psx = ctx.enter_context(tc.tile_pool(name="psx", bufs=1, space="PSUM"))
        pss = ctx.enter_context(tc.tile_pool(name="pss", bufs=1, space="PSUM"))
        psa = ctx.enter_context(tc.tile_pool(name="psa", bufs=1, space="PSUM"))

        wt0 = sb1.tile([128, C_OUT], F32R, name="wt0")
        wt1 = sb1.tile([128, C_OUT], F32R, name="wt1")
        nc.gpsimd.dma_start(out=wt0, in_=WT[0:128, :])
        nc.gpsimd.dma_start(out=wt1, in_=WT[128:256, :])
        bias = sb1.tile([C_OUT, 1], F32, name="bias")
        nc.gpsimd.dma_start(out=bias, in_=BIAS[:])
        ml = sb1.tile([128, 128], F16, name="ml")
        mr = sb1.tile([128, 512], F16, name="mr")
        nc.gpsimd.dma_start(out=ml, in_=ML[:])
        nc.gpsimd.dma_start(out=mr, in_=MR[:])
        ident = sb1.tile([128, 128], F16, name="ident")
        make_identity(nc, ident)
        ones32 = sb1.tile([128, 32], F16, name="ones32")
        nc.vector.memset(ones32, 1.0)

        for (h0, hcnt) in _superchunks():
            n = hcnt * WW
            nch = _chunks(hcnt)
            nlast = len(nch) - 1
            ncols = 128 * len(nch)
            xx = sbx.tile([128, 4 * 512], F32R, tag="xx")
            xxv = xx.rearrange("c (b k m) -> c b k m", b=2, k=2)
            nc.sync.dma_start(
                out=xxv[:, :, :, :n],
                in_=X[:, :, h0:h0 + hcnt, :].rearrange(
                    "b (k c) h w -> c b k (h w)", k=2
                ),
            )
            o = sbo.tile([C_OUT, 2 * 512], F32, tag="o")
            for bi in range(B_LOC):
                xb = 1024 * bi
                pfeat = psf.tile([C_OUT, 512], F32, tag="pfeat")
                nc.tensor.matmul(pfeat[:, :n], wt0, xx[:, xb:xb + n],
                                 start=True, stop=False)
                nc.tensor.matmul(pfeat[:, :n], wt1, xx[:, xb + 512:xb + 512 + n],
                                 start=False, stop=True)
                feat = sbf.tile([C_OUT, 512], F16, tag="feat")
                nc.vector.tensor_scalar(
                    out=feat[:, :n], in0=pfeat[:, :n],
                    scalar1=bias, scalar2=None, op0=AluOpType.add,
                )
                if n < 512:
                    nc.vector.memset(feat[:, n:], 0.0)

                xT = psx.tile([128, 512], F16, tag="xT")
                for ci, (c0, hc) in enumerate(nch):
                    nc.tensor.transpose(
                        xT[:PC, 128 * ci:128 * ci + 128],
                        feat[:, c0 * WW:c0 * WW + PC], ident,
                    )
                texp = sba.tile([PC, 512], F16, tag="texp")
                nc.scalar.activation(out=texp[:, :ncols], in_=xT[:PC, :ncols],
                                     func=EXP)
                u1 = sba.tile([PC, 512], F16, tag="u1")
                nc.vector.tensor_scalar(out=u1[:, :ncols], in0=texp[:, :ncols],
                                        scalar1=1.0, scalar2=-1.0,
                                        op0=AluOpType.min, op1=AluOpType.add)
                val = sba.tile([PC, 512], F16, tag="val")
                nc.vector.scalar_tensor_tensor(out=val[:, :ncols],
                                               in0=xT[:PC, :ncols],
                                               scalar=0.0, in1=u1[:, :ncols],
                                               op0=AluOpType.max,
                                               op1=AluOpType.add)

                ssc = pss.tile([128, 2048], F32, tag="ssc")
                eall = sba.tile([128, 2048], F16, tag="eall")
                for g in range(G):
                    r0 = 32 * g
                    nc.tensor.matmul(
                        ssc[:, 512 * g:512 * g + 504],
                        ml[r0:r0 + 15, :], mr[r0:r0 + 15, :504],
                        start=True, stop=False, tile_position=(r0, 0),
                    )
                    for ci, (c0, hc) in enumerate(nch):
                        p = hc * WW
                        col = 512 * g + 126 * ci
                        nc.tensor.matmul(
                            ssc[:, col:col + p],
                            feat[r0:r0 + 32, c0 * WW:c0 * WW + 128],
                            feat[r0:r0 + 32, c0 * WW:c0 * WW + p],
                            start=False, stop=(ci == nlast),
                            tile_position=(r0, 0),
                        )
                nc.scalar.activation(
                    out=eall.rearrange("q (g m) -> q g m", m=512)[:, :, :504],
                    in_=ssc.rearrange("q (g m) -> q g m", m=512)[:, :, :504],
                    func=EXP, scale=SCALE,
                )

                av = psa.tile([C_OUT, 512], F32, tag="av")
                for ci, (c0, hc) in enumerate(nch):
                    p = hc * WW
                    for g in range(G):
                        r0 = 32 * g
                        esl = eall[:p, 512 * g + 126 * ci:512 * g + 126 * ci + p]
                        nc.tensor.matmul(
                            av[r0:r0 + 32, c0 * WW:c0 * WW + p],
                            val[:p, 128 * ci + r0:128 * ci + r0 + 32], esl,
                            start=True, stop=True, tile_position=(0, r0),
                        )

                ev4 = eall.rearrange("q (g m) -> q g m", m=512)[:, :, :504] \
                          .rearrange("q g (c v) -> q g c v", v=126)
                rq = sba.tile([128, 16], F32, tag="rq")
                if os.environ.get("NORED"):
                    nc.vector.memset(rq, 1.0)
                else:
                    nc.vector.tensor_reduce(
                        out=rq.rearrange("q (g c) -> q g c", g=4), in_=ev4,
                        axis=mybir.AxisListType.X, op=mybir.AluOpType.add,
                    )
                rqr = sba.tile([128, 16], F32, tag="rqr")
                if os.environ.get("NORECIP"):
                    nc.vector.memset(rqr, 1.0)
                else:
                    nc.vector.reciprocal(out=rqr, in_=rq)
                rqrw = sba.tile([128, 512], F16, tag="rqrw")
                if os.environ.get("NOBCAST"):
                    nc.vector.memset(rqrw, 1.0)
                else:
                    nc.vector.tensor_scalar(
                        out=rqrw.rearrange("q (c g d) -> q c g d", c=4, g=4),
                        in0=rqr.rearrange("q (g c) -> q c g", g=4)
                        .unsqueeze(-1).broadcast_to([128, 4, 4, 32]),
                        scalar1=0.0, scalar2=None, op0=AluOpType.add,
                    )
                if not os.environ.get("NOEXP"):
                    rrx = psf.tile([C_OUT, 512], F32, tag="pfeat")
                    for ci in range(len(nch)):
                        nc.tensor.matmul(
                            rrx[:, 126 * ci:126 * ci + 126],
                            rqrw[:126, 128 * ci:128 * ci + 128],
                            ident[:126, :126],
                            start=True, stop=True,
                        )
                else:
                    rrx = av
                oslice = o[:, 512 * bi:512 * bi + n]
                nc.vector.tensor_scalar(out=oslice, in0=av[:, :n],
                                        scalar1=0.0, scalar2=None,
                                        op0=AluOpType.add)
                nc.vector.scalar_tensor_tensor(
                    out=oslice, in0=oslice, scalar=0.0, in1=rrx[:, :n],
                    op0=AluOpType.add, op1=AluOpType.mult,
                )

            nc.sync.dma_start(
                out=OUT[:, :, h0:h0 + hcnt, :].rearrange("b c h w -> c b (h w)"),
                in_=o.rearrange("c (b m) -> c b m", b=2)[:, :, :n],
            )

    return nc


_CACHED = {}


def _get_nc():
    if "nc" not in _CACHED:
        _CACHED["nc"] = _build_kernel()
    return _CACHED["nc"]


def _make_in_maps(inputs) -> list:
    x = np.asarray(inputs["neighbr_feats"], dtype=np.float32)
    w = np.asarray(inputs["W"], dtype=np.float32)
    b = np.asarray(inputs["b"], dtype=np.float32)

    wt = np.ascontiguousarray(w.T)
    bias = np.ascontiguousarray(b.reshape(C_OUT, 1))
    maskL, maskR = _host_consts()

    in_maps = []
    for core in range(N_CORES):
        xs = np.ascontiguousarray(x[core * B_LOC:(core + 1) * B_LOC])
        in_maps.append({
            "X": xs, "WT": wt, "BIAS": bias, "ML": maskL, "MR": maskR,
        })
    return in_maps


def kernel(**inputs) -> np.ndarray:
    nc = _get_nc()
    in_maps = _make_in_maps(inputs)
    res = run_bass_kernel_spmd(nc, in_maps, core_ids=list(range(N_CORES)))
    out = np.concatenate([r["OUT"] for r in res.results], axis=0)
    return out.astype(np.float32)


if __name__ == "__main__":
    rng = np.random.default_rng(0)
    inputs = {
        "neighbr_feats": rng.standard_normal((B, C_IN, H, WW)).astype(np.float32),
        "W": (rng.standard_normal((C_OUT, C_IN)) * 0.05).astype(np.float32),
        "b": (rng.standard_normal((C_OUT,)) * 0.05).astype(np.float32),
    }
    out = kernel(**inputs)
    print("kernel ran:", out.shape, out.dtype)


# revision 19
# speedup vs baseline: 1.9587x; 1.0439x over previous
import os
import sys

sys.path.insert(0, "/opt/trn_rl_repo")

import numpy as np

import concourse.bass as bass
import concourse.tile as tile
from concourse import mybir
from concourse.alu_op_type import AluOpType
from concourse.bass_utils import run_bass_kernel_spmd
from concourse.masks import make_identity

import orjson


def _split_multiwait(d: dict) -> dict:
    cnt = 0
    for fn in d.get("functions", []):
        for blk in fn.get("blocks", []):
            insts = blk.get("instructions", [])
            if not any(
                len((i.get("sync_info") or {}).get("on_wait") or []) > 1
                for i in insts
            ):
                continue
            new = []
            for ins in insts:
                si = ins.get("sync_info") or {}
                waits = si.get("on_wait") or []
                if len(waits) > 1:
                    for w in waits[:-1]:
                        cnt += 1
                        nop = {
                            "engine": ins["engine"],
                            "ins": [],
                            "outs": [],
                            "name": f"I-waitsplit-{cnt}",
                            "opcode": "NoOp",
                            "sync_info": {"on_update": [], "on_wait": [w]},
                            "text_hint": "wait_split",
                        }
                        if "debug" in ins:
                            nop["debug"] = ins["debug"]
                        new.append(nop)
                    si["on_wait"] = [waits[-1]]
                new.append(ins)
            blk["instructions"] = new
    return d


_orig_to_json_bytes = bass.Bass.to_json_bytes


def _patched_to_json_bytes(self) -> bytes:
    return orjson.dumps(_split_multiwait(orjson.loads(_orig_to_json_bytes(self))))


bass.Bass.to_json_bytes = _patched_to_json_bytes

N_CORES = 8
B, C_IN, H, WW = 16, 256, 2000, 9
C_OUT = 128
B_LOC = B // N_CORES
G, D = 4, 32
SCALE = 1.0 / np.sqrt(32.0)
MASK_A = float(np.sqrt(245.25))
MASK_S = float(np.sqrt(200.0))
HC = 14
PC = HC * WW
SUPER_H = 56
F16 = mybir.dt.float16
F32 = mybir.dt.float32
F32R = mybir.dt.float32r
EXP = mybir.ActivationFunctionType.Exp


def _superchunks():
    out = []
    h0 = 0
    while h0 + SUPER_H <= H:
        out.append((h0, SUPER_H))
        h0 += SUPER_H
    if h0 < H:
        out.append((h0, H - h0))
    return out


def _chunks(hcnt):
    out = []
    c0 = 0
    while c0 < hcnt:
        out.append((c0, min(HC, hcnt - c0)))
        c0 += HC
    return out


def _host_consts():
    maskL = np.zeros((128, 128), dtype=np.float16)
    maskR = np.zeros((128, 512), dtype=np.float16)
    for g in range(G):
        r0 = 32 * g
        maskL[r0, :PC] = MASK_A
        for ci in range(4):
            cb = 126 * ci
            maskR[r0, cb:cb + PC] = -MASK_A
            for hb in range(HC):
                maskL[r0 + 1 + hb, hb * WW:(hb + 1) * WW] = MASK_S
                maskR[r0 + 1 + hb, cb + hb * WW:cb + (hb + 1) * WW] = MASK_S
    return maskL, maskR


def _build_kernel():
    nc = bass.Bass("TRN2")
    X = nc.declare_dram_parameter("X", [B_LOC, C_IN, H, WW], F32R, isOutput=False)
    WT = nc.declare_dram_parameter("WT", [C_IN, C_OUT], F32R, isOutput=False)
    BIAS = nc.declare_dram_parameter("BIAS", [C_OUT, 1], F32, isOutput=False)
    ML = nc.declare_dram_parameter("ML", [128, 128], F16, isOutput=False)
    MR = nc.declare_dram_parameter("MR", [128, 512], F16, isOutput=False)
    OUT = nc.declare_dram_parameter("OUT", [B_LOC, C_OUT, H, WW], F32, isOutput=True)

    from contextlib import ExitStack
    with tile.TileContext(nc) as tc, ExitStack() as ctx:
        sb1 = ctx.enter_context(tc.tile_pool(name="sb1", bufs=1))
        sbx = ctx.enter_context(tc.tile_pool(name="sbx", bufs=2))
        sbf = ctx.enter_context(tc.tile_pool(name="sbf", bufs=2))
        sbo = ctx.enter_context(tc.tile_pool(name="sbo", bufs=2))
        sba = ctx.enter_context(tc.tile_pool(name="sba", bufs=2))
        psf = ctx.enter_context(tc.tile_pool(name="psf", bufs=2, space="PSUM"))
        psx = ctx.enter_context(tc.tile_pool(name="psx", bufs=1, space="PSUM"))
        pss = ctx.enter_context(tc.tile_pool(name="pss", bufs=1, space="PSUM"))
        psa = ctx.enter_context(tc.tile_pool(name="psa", bufs=1, space="PSUM"))

        wt0 = sb1.tile([128, C_OUT], F32R, name="wt0")
        wt1 = sb1.tile([128, C_OUT], F32R, name="wt1")
        nc.gpsimd.dma_start(out=wt0, in_=WT[0:128, :])
        nc.gpsimd.dma_start(out=wt1, in_=WT[128:256, :])
        bias = sb1.tile([C_OUT, 1], F32, name="bias")
        nc.gpsimd.dma_start(out=bias, in_=BIAS[:])
        ml = sb1.tile([128, 128], F16, name="ml")
        mr = sb1.tile([128, 512], F16, name="mr")
        nc.gpsimd.dma_start(out=ml, in_=ML[:])
        nc.gpsimd.dma_start(out=mr, in_=MR[:])
        ident = sb1.tile([128, 128], F16, name="ident")
        make_identity(nc, ident)
        ones32 = sb1.tile([128, 32], F16, name="ones32")
        nc.vector.memset(ones32, 1.0)

        for (h0, hcnt) in _superchunks():
            n = hcnt * WW
            nch = _chunks(hcnt)
            nlast = len(nch) - 1
            ncols = 128 * len(nch)
            xx = sbx.tile([128, 4 * 512], F32R, tag="xx")
            xxv = xx.rearrange("c (b k m) -> c b k m", b=2, k=2)
            nc.sync.dma_start(
                out=xxv[:, :, :, :n],
                in_=X[:, :, h0:h0 + hcnt, :].rearrange(
                    "b (k c) h w -> c b k (h w)", k=2
                ),
            )
            o = sbo.tile([C_OUT, 2 * 512], F32, tag="o")
            for bi in range(B_LOC):
                xb = 1024 * bi
                pfeat = psf.tile([C_OUT, 512], F32, tag="pfeat")
                nc.tensor.matmul(pfeat[:, :n], wt0, xx[:, xb:xb + n],
                                 start=True, stop=False)
                nc.tensor.matmul(pfeat[:, :n], wt1, xx[:, xb + 512:xb + 512 + n],
                                 start=False, stop=True)
                feat = sbf.tile([C_OUT, 512], F16, tag="feat")
                nc.vector.tensor_scalar(
                    out=feat[:, :n], in0=pfeat[:, :n],
                    scalar1=bias, scalar2=None, op0=AluOpType.add,
                )
                if n < 512:
                    nc.vector.memset(feat[:, n:], 0.0)

                xT = psx.tile([128, 512], F16, tag="xT")
                for ci, (c0, hc) in enumerate(nch):
                    nc.tensor.transpose(
                        xT[:PC, 128 * ci:128 * ci + 128],
                        feat[:, c0 * WW:c0 * WW + PC], ident,
                    )
                texp = sba.tile([PC, 512], F16, tag="texp")
                nc.scalar.activation(out=texp[:, :ncols], in_=xT[:PC, :ncols],
                                     func=EXP)
                u1 = sba.tile([PC, 512], F16, tag="u1")
                nc.vector.tensor_scalar(out=u1[:, :ncols], in0=texp[:, :ncols],
                                        scalar1=1.0, scalar2=-1.0,
                                        op0=AluOpType.min, op1=AluOpType.add)
                val = sba.tile([PC, 512], F16, tag="val")
                nc.vector.scalar_tensor_tensor(out=val[:, :ncols],
                                               in0=xT[:PC, :ncols],
                                               scalar=0.0, in1=u1[:, :ncols],
                                               op0=AluOpType.max,
                                               op1=AluOpType.add)

                ssc = pss.tile([128, 2048], F32, tag="ssc")
                eall = sba.tile([128, 2048], F16, tag="eall")
                for g in range(G):
                    r0 = 32 * g
                    nc.tensor.matmul(
                        ssc[:, 512 * g:512 * g + 504],
                        ml[r0:r0 + 15, :], mr[r0:r0 + 15, :504],
                        start=True, stop=False, tile_position=(r0, 0),
                    )
                    for ci, (c0, hc) in enumerate(nch):
                        p = hc * WW
                        col = 512 * g + 126 * ci
                        nc.tensor.matmul(
                            ssc[:, col:col + p],
                            feat[r0:r0 + 32, c0 * WW:c0 * WW + 128],
                            feat[r0:r0 + 32, c0 * WW:c0 * WW + p],
                            start=False, stop=(ci == nlast),
                            tile_position=(r0, 0),
                        )
                nc.scalar.activation(
                    out=eall.rearrange("q (g c v) -> q g c v", g=4, c=4)[
                        :, :, :, :126],
                    in_=ssc.rearrange("q (g m) -> q g m", m=512)[:, :, :504]
                    .rearrange("q g (c v) -> q g c v", v=126),
                    func=EXP, scale=SCALE,
                )

                av = psa.tile([C_OUT, 512], F32, tag="av")
                for ci, (c0, hc) in enumerate(nch):
                    p = hc * WW
                    for g in range(G):
                        r0 = 32 * g
                        esl = eall[:p, 512 * g + 128 * ci:512 * g + 128 * ci + p]
                        nc.tensor.matmul(
                            av[r0:r0 + 32, c0 * WW:c0 * WW + p],
                            val[:p, 128 * ci + r0:128 * ci + r0 + 32], esl,
                            start=True, stop=True, tile_position=(0, r0),
                        )

                rq = sba.tile([128, 16], F32, tag="rq")
                nc.vector.tensor_reduce(
                    out=rq,
                    in_=eall.rearrange("q (s v) -> q s v", v=128)[:, :, :126],
                    axis=mybir.AxisListType.X, op=mybir.AluOpType.add,
                )
                rqr = sba.tile([128, 16], F16, tag="rqr")
                with nc.allow_low_precision("softmax weights are fp16 anyway"):
                    nc.vector.reciprocal(out=rqr, in_=rq)
                rqrw = sba.tile([128, 512], F16, tag="rqrw")
                nc.vector.tensor_scalar(
                    out=rqrw.rearrange("q (c g d) -> q c g d", c=4, g=4),
                    in0=rqr.rearrange("q (g c) -> q c g", g=4)
                    .unsqueeze(-1).broadcast_to([128, 4, 4, 32]),
                    scalar1=0.0, scalar2=None, op0=AluOpType.add,
                )
                rrx = psf.tile([C_OUT, 512], F32, tag="pfeat")
                for ci in range(len(nch)):
                    nc.tensor.matmul(
                        rrx[:, 126 * ci:126 * ci + 126],
                        rqrw[:126, 128 * ci:128 * ci + 128],
                        ident[:126, :126],
                        start=True, stop=True,
                    )
                oslice = o[:, 512 * bi:512 * bi + n]
                nc.scalar.activation(out=oslice, in_=av[:, :n],
                                     func=mybir.ActivationFunctionType.Copy)
                nc.vector.scalar_tensor_tensor(
                    out=oslice, in0=oslice, scalar=0.0, in1=rrx[:, :n],
                    op0=AluOpType.add, op1=AluOpType.mult,
                )

            nc.sync.dma_start(
                out=OUT[:, :, h0:h0 + hcnt, :].rearrange("b c h w -> c b (h w)"),
                in_=o.rearrange("c (b m) -> c b m", b=2)[:, :, :n],
            )

    return nc


_CACHED = {}


def _get_nc():
    if "nc" not in _CACHED:
        _CACHED["nc"] = _build_kernel()
    return _CACHED["nc"]


def _make_in_maps(inputs) -> list:
    x = np.asarray(inputs["neighbr_feats"], dtype=np.float32)
    w = np.asarray(inputs["W"], dtype=np.float32)
    b = np.asarray(inputs["b"], dtype=np.float32)

    wt = np.ascontiguousarray(w.T)
    bias = np.ascontiguousarray(b.reshape(C_OUT, 1))
    maskL, maskR = _host_consts()

    in_maps = []
    for core in range(N_CORES):
        xs = np.ascontiguousarray(x[core * B_LOC:(core + 1) * B_LOC])
        in_maps.append({
            "X": xs, "WT": wt, "BIAS": bias, "ML": maskL, "MR": maskR,
        })
    return in_maps


def kernel(**inputs) -> np.ndarray:
    nc = _get_nc()
    in_maps = _make_in_maps(inputs)
    res = run_bass_kernel_spmd(nc, in_maps, core_ids=list(range(N_CORES)))
    out = np.concatenate([r["OUT"] for r in res.results], axis=0)
    return out.astype(np.float32)


if __name__ == "__main__":
    rng = np.random.default_rng(0)
    inputs = {
        "neighbr_feats": rng.standard_normal((B, C_IN, H, WW)).astype(np.float32),
        "W": (rng.standard_normal((C_OUT, C_IN)) * 0.05).astype(np.float32),
        "b": (rng.standard_normal((C_OUT,)) * 0.05).astype(np.float32),
    }
    out = kernel(**inputs)
    print("kernel ran:", out.shape, out.dtype)


# revision 20
# speedup vs baseline: 2.8945x; 1.4778x over previous
import os
import sys

sys.path.insert(0, "/opt/trn_rl_repo")

import numpy as np

import concourse.bass as bass
import concourse.tile as tile
from concourse import mybir
from concourse.alu_op_type import AluOpType
from concourse.bass_utils import run_bass_kernel_spmd
from concourse.masks import make_identity

import orjson


def _split_multiwait(d: dict) -> dict:
    cnt = 0
    for fn in d.get("functions", []):
        for blk in fn.get("blocks", []):
            insts = blk.get("instructions", [])
            if not any(
                len((i.get("sync_info") or {}).get("on_wait") or []) > 1
                for i in insts
            ):
                continue
            new = []
            for ins in insts:
                si = ins.get("sync_info") or {}
                waits = si.get("on_wait") or []
                if len(waits) > 1:
                    for w in waits[:-1]:
                        cnt += 1
                        nop = {
                            "engine": ins["engine"],
                            "ins": [],
                            "outs": [],
                            "name": f"I-waitsplit-{cnt}",
                            "opcode": "NoOp",
                            "sync_info": {"on_update": [], "on_wait": [w]},
                            "text_hint": "wait_split",
                        }
                        if "debug" in ins:
                            nop["debug"] = ins["debug"]
                        new.append(nop)
                    si["on_wait"] = [waits[-1]]
                new.append(ins)
            blk["instructions"] = new
    return d


_orig_to_json_bytes = bass.Bass.to_json_bytes


def _patched_to_json_bytes(self) -> bytes:
    return orjson.dumps(_split_multiwait(orjson.loads(_orig_to_json_bytes(self))))


bass.Bass.to_json_bytes = _patched_to_json_bytes

N_CORES = 8
B, C_IN, H, WW = 16, 256, 2000, 9
C_OUT = 128
B_LOC = B // N_CORES
G, D = 4, 32
SCALE = 1.0 / np.sqrt(32.0)
MASK_A = float(np.sqrt(245.25))
MASK_S = float(np.sqrt(200.0))
HC = 14
PC = HC * WW
SUPER_H = 56
F16 = mybir.dt.float16
F32 = mybir.dt.float32
F32R = mybir.dt.float32r
EXP = mybir.ActivationFunctionType.Exp


def _superchunks():
    out = []
    h0 = 0
    while h0 + SUPER_H <= H:
        out.append((h0, SUPER_H))
        h0 += SUPER_H
    if h0 < H:
        out.append((h0, H - h0))
    return out


def _chunks(hcnt):
    out = []
    c0 = 0
    while c0 < hcnt:
        out.append((c0, min(HC, hcnt - c0)))
        c0 += HC
    return out


def _host_consts():
    maskL = np.zeros((128, 128), dtype=np.float16)
    maskR = np.zeros((128, 512), dtype=np.float16)
    for g in range(G):
        r0 = 32 * g
        maskL[r0, :PC] = MASK_A
        for ci in range(4):
            cb = 126 * ci
            maskR[r0, cb:cb + PC] = -MASK_A
            for hb in range(HC):
                maskL[r0 + 1 + hb, hb * WW:(hb + 1) * WW] = MASK_S
                maskR[r0 + 1 + hb, cb + hb * WW:cb + (hb + 1) * WW] = MASK_S
    return maskL, maskR


def _build_kernel():
    nc = bass.Bass("TRN2")
    X = nc.declare_dram_parameter("X", [B_LOC, C_IN, H, WW], F32R, isOutput=False)
    WT = nc.declare_dram_parameter("WT", [C_IN, C_OUT], F32R, isOutput=False)
    BIAS = nc.declare_dram_parameter("BIAS", [C_OUT, 1], F32, isOutput=False)
    ML = nc.declare_dram_parameter("ML", [128, 128], F16, isOutput=False)
    MR = nc.declare_dram_parameter("MR", [128, 512], F16, isOutput=False)
    OUT = nc.declare_dram_parameter("OUT", [B_LOC, C_OUT, H, WW], F32, isOutput=True)

    from contextlib import ExitStack
    with tile.TileContext(nc) as tc, ExitStack() as ctx:
        sb1 = ctx.enter_context(tc.tile_pool(name="sb1", bufs=1))
        sbx = ctx.enter_context(tc.tile_pool(name="sbx", bufs=2))
        sbf = ctx.enter_context(tc.tile_pool(name="sbf", bufs=2))
        sbo = ctx.enter_context(tc.tile_pool(name="sbo", bufs=2))
        sba = ctx.enter_context(tc.tile_pool(name="sba", bufs=2))
        psf = ctx.enter_context(tc.tile_pool(name="psf", bufs=2, space="PSUM"))
        psx = ctx.enter_context(tc.tile_pool(name="psx", bufs=1, space="PSUM"))
        pss = ctx.enter_context(tc.tile_pool(name="pss", bufs=1, space="PSUM"))
        psa = ctx.enter_context(tc.tile_pool(name="psa", bufs=1, space="PSUM"))

        wt0 = sb1.tile([128, C_OUT], F32R, name="wt0")
        wt1 = sb1.tile([128, C_OUT], F32R, name="wt1")
        nc.gpsimd.dma_start(out=wt0, in_=WT[0:128, :])
        nc.gpsimd.dma_start(out=wt1, in_=WT[128:256, :])
        bias = sb1.tile([C_OUT, 1], F32, name="bias")
        nc.gpsimd.dma_start(out=bias, in_=BIAS[:])
        ml = sb1.tile([128, 128], F16, name="ml")
        mr = sb1.tile([128, 512], F16, name="mr")
        nc.gpsimd.dma_start(out=ml, in_=ML[:])
        nc.gpsimd.dma_start(out=mr, in_=MR[:])
        ident = sb1.tile([128, 128], F16, name="ident")
        make_identity(nc, ident)
        ones32 = sb1.tile([128, 32], F16, name="ones32")
        nc.vector.memset(ones32, 1.0)

        def head(h0, hcnt, bi, xx, o):
            n = hcnt * WW
            nch = _chunks(hcnt)
            nlast = len(nch) - 1
            ncols = 128 * len(nch)
            xb = 1024 * bi
            pfeat = psf.tile([C_OUT, 512], F32, tag="pfeat")
            nc.tensor.matmul(pfeat[:, :n], wt0, xx[:, xb:xb + n],
                             start=True, stop=False)
            nc.tensor.matmul(pfeat[:, :n], wt1, xx[:, xb + 512:xb + 512 + n],
                             start=False, stop=True)
            feat = sbf.tile([C_OUT, 512], F16, tag="feat")
            nc.vector.tensor_scalar(
                out=feat[:, :n], in0=pfeat[:, :n],
                scalar1=bias, scalar2=None, op0=AluOpType.add,
            )
            if n < 512:
                nc.vector.memset(feat[:, n:], 0.0)

            xT = psx.tile([128, 512], F16, tag="xT")
            for ci, (c0, hc) in enumerate(nch):
                nc.tensor.transpose(
                    xT[:PC, 128 * ci:128 * ci + 128],
                    feat[:, c0 * WW:c0 * WW + PC], ident,
                )
            texp = sba.tile([PC, 512], F16, tag="texp")
            nc.scalar.activation(out=texp[:, :ncols], in_=xT[:PC, :ncols],
                                 func=EXP)
            u1 = sba.tile([PC, 512], F16, tag="u1")
            nc.vector.tensor_scalar(out=u1[:, :ncols], in0=texp[:, :ncols],
                                    scalar1=1.0, scalar2=-1.0,
                                    op0=AluOpType.min, op1=AluOpType.add)
            val = sba.tile([PC, 512], F16, tag="val")
            nc.vector.scalar_tensor_tensor(out=val[:, :ncols],
                                           in0=xT[:PC, :ncols],
                                           scalar=0.0, in1=u1[:, :ncols],
                                           op0=AluOpType.max,
                                           op1=AluOpType.add)

            ssc = pss.tile([128, 2048], F32, tag="ssc")
            eall = sba.tile([128, 2048], F16, tag="eall")
            for g in range(G):
                r0 = 32 * g
                nc.tensor.matmul(
                    ssc[:, 512 * g:512 * g + 504],
                    ml[r0:r0 + 15, :], mr[r0:r0 + 15, :504],
                    start=True, stop=False, tile_position=(r0, 0),
                )
                for ci, (c0, hc) in enumerate(nch):
                    p = hc * WW
                    col = 512 * g + 126 * ci
                    nc.tensor.matmul(
                        ssc[:, col:col + p],
                        feat[r0:r0 + 32, c0 * WW:c0 * WW + 128],
                        feat[r0:r0 + 32, c0 * WW:c0 * WW + p],
                        start=False, stop=(ci == nlast),
                        tile_position=(r0, 0),
                    )
            nc.scalar.activation(
                out=eall.rearrange("q (g c v) -> q g c v", g=4, c=4)[
                    :, :, :, :126],
                in_=ssc.rearrange("q (g m) -> q g m", m=512)[:, :, :504]
                .rearrange("q g (c v) -> q g c v", v=126),
                func=EXP, scale=SCALE,
            )
            return (hcnt, bi, val, eall, o)

        def tail(state):
            hcnt, bi, val, eall, o = state
            n = hcnt * WW
            nch = _chunks(hcnt)
            rq = sba.tile([128, 16], F32, tag="rq")
            nc.vector.tensor_reduce(
                out=rq,
                in_=eall.rearrange("q (s v) -> q s v", v=128)[:, :, :126],
                axis=mybir.AxisListType.X, op=mybir.AluOpType.add,
            )
            rqr = sba.tile([128, 16], F16, tag="rqr")
            with nc.allow_low_precision("softmax weights are fp16 anyway"):
                nc.vector.reciprocal(out=rqr, in_=rq)
            rqrw = sba.tile([128, 512], F16, tag="rqrw")
            nc.vector.tensor_scalar(
                out=rqrw.rearrange("q (c g d) -> q c g d", c=4, g=4),
                in0=rqr.rearrange("q (g c) -> q c g", g=4)
                .unsqueeze(-1).broadcast_to([128, 4, 4, 32]),
                scalar1=0.0, scalar2=None, op0=AluOpType.add,
            )
            rrx = psf.tile([C_OUT, 512], F32, tag="pfeat")
            for ci in range(len(nch)):
                nc.tensor.matmul(
                    rrx[:, 126 * ci:126 * ci + 126],
                    rqrw[:126, 128 * ci:128 * ci + 128],
                    ident[:126, :126],
                    start=True, stop=True,
                )
            av = psa.tile([C_OUT, 512], F32, tag="av")
            for ci, (c0, hc) in enumerate(nch):
                p = hc * WW
                for g in range(G):
                    r0 = 32 * g
                    esl = eall[:p, 512 * g + 128 * ci:512 * g + 128 * ci + p]
                    nc.tensor.matmul(
                        av[r0:r0 + 32, c0 * WW:c0 * WW + p],
                        val[:p, 128 * ci + r0:128 * ci + r0 + 32], esl,
                        start=True, stop=True, tile_position=(0, r0),
                    )
            oslice = o[:, 512 * bi:512 * bi + n]
            nc.scalar.activation(out=oslice, in_=av[:, :n],
                                 func=mybir.ActivationFunctionType.Copy)
            nc.vector.scalar_tensor_tensor(
                out=oslice, in0=oslice, scalar=0.0, in1=rrx[:, :n],
                op0=AluOpType.add, op1=AluOpType.mult,
            )

        scs = _superchunks()

        def issue_in_dma(k):
            h0, hcnt = scs[k]
            n = hcnt * WW
            xx = sbx.tile([128, 4 * 512], F32R, tag="xx")
            xxv = xx.rearrange("c (b k m) -> c b k m", b=2, k=2)
            nc.sync.dma_start(
                out=xxv[:, :, :, :n],
                in_=X[:, :, h0:h0 + hcnt, :].rearrange(
                    "b (k c) h w -> c b k (h w)", k=2
                ),
            )
            return xx

        def issue_out_dma(k, o):
            h0, hcnt = scs[k]
            n = hcnt * WW
            nc.sync.dma_start(
                out=OUT[:, :, h0:h0 + hcnt, :].rearrange(
                    "b c h w -> c b (h w)"),
                in_=o.rearrange("c (b m) -> c b m", b=2)[:, :, :n],
            )

        pending = []
        xx = issue_in_dma(0)
        for k, (h0, hcnt) in enumerate(scs):
            o = sbo.tile([C_OUT, 2 * 512], F32, tag="o")
            for bi in range(B_LOC):
                if k + 1 < len(scs) and bi == 1:
                    nxx = issue_in_dma(k + 1)
                else:
                    nxx = None
                st = head(h0, hcnt, bi, xx, o)
                if pending:
                    pst, pk, po, pbi = pending.pop(0)
                    tail(pst)
                    if pbi == 1:
                        issue_out_dma(pk, po)
                pending.append((st, k, o, bi))
                if nxx is not None:
                    xx = nxx
        while pending:
            pst, pk, po, pbi = pending.pop(0)
            tail(pst)
            if pbi == 1:
                issue_out_dma(pk, po)

    return nc


_CACHED = {}


def _get_nc():
    if "nc" not in _CACHED:
        _CACHED["nc"] = _build_kernel()
    return _CACHED["nc"]


def _make_in_maps(inputs) -> list:
    x = np.asarray(inputs["neighbr_feats"], dtype=np.float32)
    w = np.asarray(inputs["W"], dtype=np.float32)
    b = np.asarray(inputs["b"], dtype=np.float32)

    wt = np.ascontiguousarray(w.T)
    bias = np.ascontiguousarray(b.reshape(C_OUT, 1))
    maskL, maskR = _host_consts()

    in_maps = []
    for core in range(N_CORES):
        xs = np.ascontiguousarray(x[core * B_LOC:(core + 1) * B_LOC])
        in_maps.append({
            "X": xs, "WT": wt, "BIAS": bias, "ML": maskL, "MR": maskR,
        })
    return in_maps


def kernel(**inputs) -> np.ndarray:
    nc = _get_nc()
    in_maps = _make_in_maps(inputs)
    res = run_bass_kernel_spmd(nc, in_maps, core_ids=list(range(N_CORES)))
    out = np.concatenate([r["OUT"] for r in res.results], axis=0)
    return out.astype(np.float32)


if __name__ == "__main__":
    rng = np.random.default_rng(0)
    inputs = {
        "neighbr_feats": rng.standard_normal((B, C_IN, H, WW)).astype(np.float32),
        "W": (rng.standard_normal((C_OUT, C_IN)) * 0.05).astype(np.float32),
        "b": (rng.standard_normal((C_OUT,)) * 0.05).astype(np.float32),
    }
    out = kernel(**inputs)
    print("kernel ran:", out.shape, out.dtype)


# revision 21
# speedup vs baseline: 2.9071x; 1.0044x over previous
import os
import sys

sys.path.insert(0, "/opt/trn_rl_repo")

import numpy as np

import concourse.bass as bass
import concourse.tile as tile
from concourse import mybir
from concourse.alu_op_type import AluOpType
from concourse.bass_utils import run_bass_kernel_spmd
from concourse.masks import make_identity

import orjson


def _split_multiwait(d: dict) -> dict:
    cnt = 0
    for fn in d.get("functions", []):
        for blk in fn.get("blocks", []):
            insts = blk.get("instructions", [])
            if not any(
                len((i.get("sync_info") or {}).get("on_wait") or []) > 1
                for i in insts
            ):
                continue
            new = []
            for ins in insts:
                si = ins.get("sync_info") or {}
                waits = si.get("on_wait") or []
                if len(waits) > 1:
                    for w in waits[:-1]:
                        cnt += 1
                        nop = {
                            "engine": ins["engine"],
                            "ins": [],
                            "outs": [],
                            "name": f"I-waitsplit-{cnt}",
                            "opcode": "NoOp",
                            "sync_info": {"on_update": [], "on_wait": [w]},
                            "text_hint": "wait_split",
                        }
                        if "debug" in ins:
                            nop["debug"] = ins["debug"]
                        new.append(nop)
                    si["on_wait"] = [waits[-1]]
                new.append(ins)
            blk["instructions"] = new
    return d


_orig_to_json_bytes = bass.Bass.to_json_bytes


def _patched_to_json_bytes(self) -> bytes:
    return orjson.dumps(_split_multiwait(orjson.loads(_orig_to_json_bytes(self))))


bass.Bass.to_json_bytes = _patched_to_json_bytes

N_CORES = 8
B, C_IN, H, WW = 16, 256, 2000, 9
C_OUT = 128
B_LOC = B // N_CORES
G, D = 4, 32
SCALE = 1.0 / np.sqrt(32.0)
MASK_A = float(np.sqrt(245.25))
MASK_S = float(np.sqrt(200.0))
HC = 14
PC = HC * WW
SUPER_H = 56
F16 = mybir.dt.float16
F32 = mybir.dt.float32
F32R = mybir.dt.float32r
EXP = mybir.ActivationFunctionType.Exp


def _superchunks():
    out = []
    h0 = 0
    while h0 + SUPER_H <= H:
        out.append((h0, SUPER_H))
        h0 += SUPER_H
    if h0 < H:
        out.append((h0, H - h0))
    return out


def _chunks(hcnt):
    out = []
    c0 = 0
    while c0 < hcnt:
        out.append((c0, min(HC, hcnt - c0)))
        c0 += HC
    return out


def _host_consts():
    maskL = np.zeros((128, 128), dtype=np.float16)
    maskR = np.zeros((128, 512), dtype=np.float16)
    for g in range(G):
        r0 = 32 * g
        maskL[r0, :PC] = MASK_A
        maskR[r0, :] = -MASK_A
        for ci in range(4):
            cb = 128 * ci
            for hb in range(HC):
                maskL[r0 + 1 + hb, hb * WW:(hb + 1) * WW] = MASK_S
                maskR[r0 + 1 + hb, cb + hb * WW:cb + (hb + 1) * WW] = MASK_S
    return maskL, maskR


def _build_kernel():
    nc = bass.Bass("TRN2")
    X = nc.declare_dram_parameter("X", [B_LOC, C_IN, H, WW], F32, isOutput=False)
    WT = nc.declare_dram_parameter("WT", [C_IN, C_OUT], F16, isOutput=False)
    BIAS = nc.declare_dram_parameter("BIAS", [C_OUT, 1], F32, isOutput=False)
    ML = nc.declare_dram_parameter("ML", [128, 128], F16, isOutput=False)
    MR = nc.declare_dram_parameter("MR", [128, 512], F16, isOutput=False)
    OUT = nc.declare_dram_parameter("OUT", [B_LOC, C_OUT, H, WW], F32, isOutput=True)

    from contextlib import ExitStack
    with tile.TileContext(nc) as tc, ExitStack() as ctx:
        sb1 = ctx.enter_context(tc.tile_pool(name="sb1", bufs=1))
        sbx = ctx.enter_context(tc.tile_pool(name="sbx", bufs=2))
        sbf = ctx.enter_context(tc.tile_pool(name="sbf", bufs=2))
        sbo = ctx.enter_context(tc.tile_pool(name="sbo", bufs=2))
        sba = ctx.enter_context(tc.tile_pool(name="sba", bufs=2))
        psf = ctx.enter_context(tc.tile_pool(name="psf", bufs=2, space="PSUM"))
        psx = ctx.enter_context(tc.tile_pool(name="psx", bufs=1, space="PSUM"))
        pss = ctx.enter_context(tc.tile_pool(name="pss", bufs=1, space="PSUM"))
        psa = ctx.enter_context(tc.tile_pool(name="psa", bufs=1, space="PSUM"))

        wt0 = sb1.tile([128, C_OUT], F16, name="wt0")
        wt1 = sb1.tile([128, C_OUT], F16, name="wt1")
        nc.gpsimd.dma_start(out=wt0, in_=WT[0:128, :])
        nc.gpsimd.dma_start(out=wt1, in_=WT[128:256, :])
        bias = sb1.tile([C_OUT, 1], F32, name="bias")
        nc.gpsimd.dma_start(out=bias, in_=BIAS[:])
        ml = sb1.tile([128, 128], F16, name="ml")
        mr = sb1.tile([128, 512], F16, name="mr")
        nc.gpsimd.dma_start(out=ml, in_=ML[:])
        nc.gpsimd.dma_start(out=mr, in_=MR[:])
        ident = sb1.tile([128, 128], F16, name="ident")
        make_identity(nc, ident)
        ones32 = sb1.tile([128, 32], F16, name="ones32")
        nc.vector.memset(ones32, 1.0)

        def head(h0, hcnt, bi, xx, o):
            n = hcnt * WW
            nch = _chunks(hcnt)
            nlast = len(nch) - 1
            ncols = 128 * len(nch)
            xb = 1024 * bi
            pfeat = psf.tile([C_OUT, 512], F32, tag="pfeat")
            nc.tensor.matmul(pfeat[:, :n], wt0, xx[:, xb:xb + n],
                             start=True, stop=False)
            nc.tensor.matmul(pfeat[:, :n], wt1, xx[:, xb + 512:xb + 512 + n],
                             start=False, stop=True)
            feat = sbf.tile([C_OUT, 512], F16, tag="feat")
            nc.vector.tensor_scalar(
                out=feat[:, :n], in0=pfeat[:, :n],
                scalar1=bias, scalar2=None, op0=AluOpType.add,
            )
            if n < 512:
                nc.vector.memset(feat[:, n:], 0.0)

            xT = psx.tile([128, 512], F16, tag="xT")
            for ci, (c0, hc) in enumerate(nch):
                nc.tensor.transpose(
                    xT[:PC, 128 * ci:128 * ci + 128],
                    feat[:, c0 * WW:c0 * WW + PC], ident,
                )
            texp = sba.tile([PC, 512], F16, tag="texp")
            nc.scalar.activation(out=texp[:, :ncols], in_=xT[:PC, :ncols],
                                 func=EXP)
            u1 = sba.tile([PC, 512], F16, tag="u1")
            nc.vector.tensor_scalar(out=u1[:, :ncols], in0=texp[:, :ncols],
                                    scalar1=1.0, scalar2=-1.0,
                                    op0=AluOpType.min, op1=AluOpType.add)
            val = sba.tile([PC, 512], F16, tag="val")
            nc.vector.scalar_tensor_tensor(out=val[:, :ncols],
                                           in0=xT[:PC, :ncols],
                                           scalar=0.0, in1=u1[:, :ncols],
                                           op0=AluOpType.max,
                                           op1=AluOpType.add)

            ssc = pss.tile([128, 2048], F32, tag="ssc")
            eall = sba.tile([128, 2048], F16, tag="eall")
            for g in range(G):
                r0 = 32 * g
                nc.tensor.matmul(
                    ssc[:, 512 * g:512 * g + 512],
                    ml[r0:r0 + 15, :], mr[r0:r0 + 15, :],
                    start=True, stop=False, tile_position=(r0, 0),
                )
                for ci, (c0, hc) in enumerate(nch):
                    p = hc * WW
                    col = 512 * g + 128 * ci
                    nc.tensor.matmul(
                        ssc[:, col:col + p],
                        feat[r0:r0 + 32, c0 * WW:c0 * WW + 128],
                        feat[r0:r0 + 32, c0 * WW:c0 * WW + p],
                        start=False, stop=(ci == nlast),
                        tile_position=(r0, 0),
                    )
            nc.scalar.activation(out=eall, in_=ssc, func=EXP, scale=SCALE)
            return (hcnt, bi, val, eall, o)

        def tail(state):
            hcnt, bi, val, eall, o = state
            n = hcnt * WW
            nch = _chunks(hcnt)
            rq = sba.tile([128, 16], F32, tag="rq")
            nc.vector.tensor_reduce(
                out=rq,
                in_=eall.rearrange("q (s v) -> q s v", v=128),
                axis=mybir.AxisListType.X, op=mybir.AluOpType.add,
            )
            rqr = sba.tile([128, 16], F16, tag="rqr")
            with nc.allow_low_precision("softmax weights are fp16 anyway"):
                nc.vector.reciprocal(out=rqr, in_=rq)
            rqrw = sba.tile([128, 512], F16, tag="rqrw")
            nc.vector.tensor_scalar(
                out=rqrw.rearrange("q (c g d) -> q c g d", c=4, g=4),
                in0=rqr.rearrange("q (g c) -> q c g", g=4)
                .unsqueeze(-1).broadcast_to([128, 4, 4, 32]),
                scalar1=0.0, scalar2=None, op0=AluOpType.add,
            )
            rrx = psf.tile([C_OUT, 512], F32, tag="pfeat")
            for ci in range(len(nch)):
                nc.tensor.matmul(
                    rrx[:, 126 * ci:126 * ci + 126],
                    rqrw[:126, 128 * ci:128 * ci + 128],
                    ident[:126, :126],
                    start=True, stop=True,
                )
            av = psa.tile([C_OUT, 512], F32, tag="av")
            for ci, (c0, hc) in enumerate(nch):
                p = hc * WW
                for g in range(G):
                    r0 = 32 * g
                    esl = eall[:p, 512 * g + 128 * ci:512 * g + 128 * ci + p]
                    nc.tensor.matmul(
                        av[r0:r0 + 32, c0 * WW:c0 * WW + p],
                        val[:p, 128 * ci + r0:128 * ci + r0 + 32], esl,
                        start=True, stop=True, tile_position=(0, r0),
                    )
            oslice = o[:, 512 * bi:512 * bi + n]
            nc.scalar.activation(out=oslice, in_=av[:, :n],
                                 func=mybir.ActivationFunctionType.Copy)
            nc.vector.scalar_tensor_tensor(
                out=oslice, in0=oslice, scalar=0.0, in1=rrx[:, :n],
                op0=AluOpType.add, op1=AluOpType.mult,
            )

        scs = _superchunks()

        def issue_in_dma(k):
            h0, hcnt = scs[k]
            n = hcnt * WW
            xx = sbx.tile([128, 4 * 512], F16, tag="xx")
            xxv = xx.rearrange("c (b k m) -> c b k m", b=2, k=2)
            nc.gpsimd.dma_start(
                out=xxv[:, :, :, :n],
                in_=X[:, :, h0:h0 + hcnt, :].rearrange(
                    "b (k c) h w -> c b k (h w)", k=2
                ),
            )
            return xx

        def issue_out_dma(k, o):
            h0, hcnt = scs[k]
            n = hcnt * WW
            nc.sync.dma_start(
                out=OUT[:, :, h0:h0 + hcnt, :].rearrange(
                    "b c h w -> c b (h w)"),
                in_=o.rearrange("c (b m) -> c b m", b=2)[:, :, :n],
            )

        pending = []
        xx = issue_in_dma(0)
        for k, (h0, hcnt) in enumerate(scs):
            o = sbo.tile([C_OUT, 2 * 512], F32, tag="o")
            for bi in range(B_LOC):
                if k + 1 < len(scs) and bi == 1:
                    nxx = issue_in_dma(k + 1)
                else:
                    nxx = None
                st = head(h0, hcnt, bi, xx, o)
                if pending:
                    pst, pk, po, pbi = pending.pop(0)
                    tail(pst)
                    if pbi == 1:
                        issue_out_dma(pk, po)
                pending.append((st, k, o, bi))
                if nxx is not None:
                    xx = nxx
        while pending:
            pst, pk, po, pbi = pending.pop(0)
            tail(pst)
            if pbi == 1:
                issue_out_dma(pk, po)

    return nc


_CACHED = {}


def _get_nc():
    if "nc" not in _CACHED:
        _CACHED["nc"] = _build_kernel()
    return _CACHED["nc"]


def _make_in_maps(inputs) -> list:
    x = np.asarray(inputs["neighbr_feats"], dtype=np.float32)
    w = np.asarray(inputs["W"], dtype=np.float32)
    b = np.asarray(inputs["b"], dtype=np.float32)

    wt = np.ascontiguousarray(w.T.astype(np.float16))
    bias = np.ascontiguousarray(b.reshape(C_OUT, 1))
    maskL, maskR = _host_consts()

    in_maps = []
    for core in range(N_CORES):
        xs = np.ascontiguousarray(x[core * B_LOC:(core + 1) * B_LOC])
        in_maps.append({
            "X": xs, "WT": wt, "BIAS": bias, "ML": maskL, "MR": maskR,
        })
    return in_maps


def kernel(**inputs) -> np.ndarray:
    nc = _get_nc()
    in_maps = _make_in_maps(inputs)
    res = run_bass_kernel_spmd(nc, in_maps, core_ids=list(range(N_CORES)))
    out = np.concatenate([r["OUT"] for r in res.results], axis=0)
    return out.astype(np.float32)


if __name__ == "__main__":
    rng = np.random.default_rng(0)
    inputs = {
        "neighbr_feats": rng.standard_normal((B, C_IN, H, WW)).astype(np.float32),
        "W": (rng.standard_normal((C_OUT, C_IN)) * 0.05).astype(np.float32),
        "b": (rng.standard_normal((C_OUT,)) * 0.05).astype(np.float32),
    }
    out = kernel(**inputs)
    print("kernel ran:", out.shape, out.dtype)


# revision 22
# speedup vs baseline: 3.4134x; 1.1742x over previous
import os
import sys

sys.path.insert(0, "/opt/trn_rl_repo")

import numpy as np

import concourse.bass as bass
import concourse.tile as tile
from concourse import mybir
from concourse.alu_op_type import AluOpType
from concourse.bass_utils import run_bass_kernel_spmd
from concourse.masks import make_identity

import orjson


def _split_multiwait(d: dict) -> dict:
    cnt = 0
    for fn in d.get("functions", []):
        for blk in fn.get("blocks", []):
            insts = blk.get("instructions", [])
            if not any(
                len((i.get("sync_info") or {}).get("on_wait") or []) > 1
                for i in insts
            ):
                continue
            new = []
            for ins in insts:
                si = ins.get("sync_info") or {}
                waits = si.get("on_wait") or []
                if len(waits) > 1:
                    for w in waits[:-1]:
                        cnt += 1
                        nop = {
                            "engine": ins["engine"],
                            "ins": [],
                            "outs": [],
                            "name": f"I-waitsplit-{cnt}",
                            "opcode": "NoOp",
                            "sync_info": {"on_update": [], "on_wait": [w]},
                            "text_hint": "wait_split",
                        }
                        if "debug" in ins:
                            nop["debug"] = ins["debug"]
                        new.append(nop)
                    si["on_wait"] = [waits[-1]]
                new.append(ins)
            blk["instructions"] = new
    return d


_orig_to_json_bytes = bass.Bass.to_json_bytes


def _patched_to_json_bytes(self) -> bytes:
    return orjson.dumps(_split_multiwait(orjson.loads(_orig_to_json_bytes(self))))


bass.Bass.to_json_bytes = _patched_to_json_bytes

N_CORES = 8
B, C_IN, H, WW = 16, 256, 2000, 9
C_OUT = 128
B_LOC = B // N_CORES
G, D = 4, 32
SCALE = 1.0 / np.sqrt(32.0)
MASK_A = float(np.sqrt(245.25))
MASK_S = float(np.sqrt(200.0))
HC = 14
PC = HC * WW
SUPER_H = 56
F16 = mybir.dt.float16
F32 = mybir.dt.float32
F32R = mybir.dt.float32r
EXP = mybir.ActivationFunctionType.Exp


def _superchunks():
    out = []
    h0 = 0
    while h0 + SUPER_H <= H:
        out.append((h0, SUPER_H))
        h0 += SUPER_H
    if h0 < H:
        out.append((h0, H - h0))
    return out


def _chunks(hcnt):
    out = []
    c0 = 0
    while c0 < hcnt:
        out.append((c0, min(HC, hcnt - c0)))
        c0 += HC
    return out


def _host_consts():
    maskL = np.zeros((128, 128), dtype=np.float16)
    maskR = np.zeros((128, 512), dtype=np.float16)
    for g in range(G):
        r0 = 32 * g
        maskL[r0, :PC] = MASK_A
        maskR[r0, :] = -MASK_A
        for ci in range(4):
            cb = 128 * ci
            for hb in range(HC):
                maskL[r0 + 1 + hb, hb * WW:(hb + 1) * WW] = MASK_S
                maskR[r0 + 1 + hb, cb + hb * WW:cb + (hb + 1) * WW] = MASK_S
    return maskL, maskR


def _build_kernel():
    nc = bass.Bass("TRN2")
    X = nc.declare_dram_parameter("X", [B_LOC, C_IN, H, WW], F32, isOutput=False)
    WT = nc.declare_dram_parameter("WT", [C_IN, C_OUT], F16, isOutput=False)
    BIAS = nc.declare_dram_parameter("BIAS", [C_OUT, 1], F32, isOutput=False)
    ML = nc.declare_dram_parameter("ML", [128, 128], F16, isOutput=False)
    MR = nc.declare_dram_parameter("MR", [128, 512], F16, isOutput=False)
    OUT = nc.declare_dram_parameter("OUT", [B_LOC, C_OUT, H, WW], F32, isOutput=True)

    from contextlib import ExitStack
    with tile.TileContext(nc) as tc, ExitStack() as ctx:
        sb1 = ctx.enter_context(tc.tile_pool(name="sb1", bufs=1))
        sbx = ctx.enter_context(tc.tile_pool(name="sbx", bufs=2))
        sbf = ctx.enter_context(tc.tile_pool(name="sbf", bufs=2))
        sbo = ctx.enter_context(tc.tile_pool(name="sbo", bufs=3))
        sba = ctx.enter_context(tc.tile_pool(name="sba", bufs=3))
        psf = ctx.enter_context(tc.tile_pool(name="psf", bufs=2, space="PSUM"))
        psx = ctx.enter_context(tc.tile_pool(name="psx", bufs=1, space="PSUM"))
        pss = ctx.enter_context(tc.tile_pool(name="pss", bufs=1, space="PSUM"))
        psa = ctx.enter_context(tc.tile_pool(name="psa", bufs=1, space="PSUM"))

        wt0 = sb1.tile([128, C_OUT], F16, name="wt0")
        wt1 = sb1.tile([128, C_OUT], F16, name="wt1")
        nc.gpsimd.dma_start(out=wt0, in_=WT[0:128, :])
        nc.gpsimd.dma_start(out=wt1, in_=WT[128:256, :])
        bias = sb1.tile([C_OUT, 1], F32, name="bias")
        nc.gpsimd.dma_start(out=bias, in_=BIAS[:])
        ml = sb1.tile([128, 128], F16, name="ml")
        mr = sb1.tile([128, 512], F16, name="mr")
        nc.gpsimd.dma_start(out=ml, in_=ML[:])
        nc.gpsimd.dma_start(out=mr, in_=MR[:])
        ident = sb1.tile([128, 128], F16, name="ident")
        make_identity(nc, ident)
        ones32 = sb1.tile([128, 32], F16, name="ones32")
        nc.vector.memset(ones32, 1.0)

        def head(h0, hcnt, bi, xx, o):
            n = hcnt * WW
            nch = _chunks(hcnt)
            nlast = len(nch) - 1
            ncols = 128 * len(nch)
            xb = 1024 * bi
            pfeat = psf.tile([C_OUT, 512], F32, tag="pfeat")
            nc.tensor.matmul(pfeat[:, :n], wt0, xx[:, xb:xb + n],
                             start=True, stop=False)
            nc.tensor.matmul(pfeat[:, :n], wt1, xx[:, xb + 512:xb + 512 + n],
                             start=False, stop=True)
            feat = sbf.tile([C_OUT, 512], F16, tag="feat")
            nc.vector.tensor_scalar(
                out=feat[:, :n], in0=pfeat[:, :n],
                scalar1=bias, scalar2=None, op0=AluOpType.add,
            )
            if n < 512:
                nc.vector.memset(feat[:, n:], 0.0)

            xT = psx.tile([128, 512], F16, tag="xT")
            for ci, (c0, hc) in enumerate(nch):
                nc.tensor.transpose(
                    xT[:PC, 128 * ci:128 * ci + 128],
                    feat[:, c0 * WW:c0 * WW + PC], ident,
                )
            texp = sba.tile([PC, 512], F16, tag="texp")
            nc.scalar.activation(out=texp[:, :ncols], in_=xT[:PC, :ncols],
                                 func=EXP)
            u1 = sba.tile([PC, 512], F16, tag="u1")
            nc.vector.tensor_scalar(out=u1[:, :ncols], in0=texp[:, :ncols],
                                    scalar1=1.0, scalar2=-1.0,
                                    op0=AluOpType.min, op1=AluOpType.add)
            val = sba.tile([PC, 512], F16, tag="val")
            nc.vector.scalar_tensor_tensor(out=val[:, :ncols],
                                           in0=xT[:PC, :ncols],
                                           scalar=0.0, in1=u1[:, :ncols],
                                           op0=AluOpType.max,
                                           op1=AluOpType.add)

            ssc = pss.tile([128, 2048], F32, tag="ssc")
            eall = sba.tile([128, 2048], F16, tag="eall")
            for g in range(G):
                r0 = 32 * g
                nc.tensor.matmul(
                    ssc[:, 512 * g:512 * g + 512],
                    ml[r0:r0 + 15, :], mr[r0:r0 + 15, :],
                    start=True, stop=False, tile_position=(r0, 0),
                )
                for ci, (c0, hc) in enumerate(nch):
                    p = hc * WW
                    col = 512 * g + 128 * ci
                    nc.tensor.matmul(
                        ssc[:, col:col + p],
                        feat[r0:r0 + 32, c0 * WW:c0 * WW + 128],
                        feat[r0:r0 + 32, c0 * WW:c0 * WW + p],
                        start=False, stop=(ci == nlast),
                        tile_position=(r0, 0),
                    )
            nc.scalar.activation(out=eall, in_=ssc, func=EXP, scale=SCALE)
            return (hcnt, bi, val, eall, o)

        def tail(state):
            hcnt, bi, val, eall, o = state
            n = hcnt * WW
            nch = _chunks(hcnt)
            rq = sba.tile([128, 16], F32, tag="rq")
            nc.vector.tensor_reduce(
                out=rq,
                in_=eall.rearrange("q (s v) -> q s v", v=128),
                axis=mybir.AxisListType.X, op=mybir.AluOpType.add,
            )
            rqr = sba.tile([128, 16], F16, tag="rqr")
            with nc.allow_low_precision("softmax weights are fp16 anyway"):
                nc.vector.reciprocal(out=rqr, in_=rq)
            rqrw = sba.tile([128, 512], F16, tag="rqrw")
            nc.vector.tensor_scalar(
                out=rqrw.rearrange("q (c g d) -> q c g d", c=4, g=4),
                in0=rqr.rearrange("q (g c) -> q c g", g=4)
                .unsqueeze(-1).broadcast_to([128, 4, 4, 32]),
                scalar1=0.0, scalar2=None, op0=AluOpType.add,
            )
            rrx = psf.tile([C_OUT, 512], F32, tag="pfeat")
            for ci in range(len(nch)):
                nc.tensor.matmul(
                    rrx[:, 126 * ci:126 * ci + 126],
                    rqrw[:126, 128 * ci:128 * ci + 128],
                    ident[:126, :126],
                    start=True, stop=True,
                )
            av = psa.tile([C_OUT, 512], F32, tag="av")
            for ci, (c0, hc) in enumerate(nch):
                p = hc * WW
                for g in range(G):
                    r0 = 32 * g
                    esl = eall[:p, 512 * g + 128 * ci:512 * g + 128 * ci + p]
                    nc.tensor.matmul(
                        av[r0:r0 + 32, c0 * WW:c0 * WW + p],
                        val[:p, 128 * ci + r0:128 * ci + r0 + 32], esl,
                        start=True, stop=True, tile_position=(0, r0),
                    )
            oslice = o[:, 512 * bi:512 * bi + n]
            nc.scalar.activation(out=oslice, in_=av[:, :n],
                                 func=mybir.ActivationFunctionType.Copy)
            nc.vector.scalar_tensor_tensor(
                out=oslice, in0=oslice, scalar=0.0, in1=rrx[:, :n],
                op0=AluOpType.add, op1=AluOpType.mult,
            )

        scs = _superchunks()

        def issue_in_dma(k):
            h0, hcnt = scs[k]
            n = hcnt * WW
            xx = sbx.tile([128, 4 * 512], F16, tag="xx")
            xxv = xx.rearrange("c (b k m) -> c b k m", b=2, k=2)
            nc.gpsimd.dma_start(
                out=xxv[:, :, :, :n],
                in_=X[:, :, h0:h0 + hcnt, :].rearrange(
                    "b (k c) h w -> c b k (h w)", k=2
                ),
            )
            return xx

        def issue_out_dma(k, o):
            h0, hcnt = scs[k]
            n = hcnt * WW
            nc.sync.dma_start(
                out=OUT[:, :, h0:h0 + hcnt, :].rearrange(
                    "b c h w -> c b (h w)"),
                in_=o.rearrange("c (b m) -> c b m", b=2)[:, :, :n],
            )

        pending = []
        xx = issue_in_dma(0)
        for k, (h0, hcnt) in enumerate(scs):
            o = sbo.tile([C_OUT, 2 * 512], F32, tag="o")
            for bi in range(B_LOC):
                if k + 1 < len(scs) and bi == 1:
                    nxx = issue_in_dma(k + 1)
                else:
                    nxx = None
                st = head(h0, hcnt, bi, xx, o)
                if len(pending) >= 2:
                    pst, pk, po, pbi = pending.pop(0)
                    tail(pst)
                    if pbi == 1:
                        issue_out_dma(pk, po)
                pending.append((st, k, o, bi))
                if nxx is not None:
                    xx = nxx
        while pending:
            pst, pk, po, pbi = pending.pop(0)
            tail(pst)
            if pbi == 1:
                issue_out_dma(pk, po)

    return nc


_CACHED = {}


def _get_nc():
    if "nc" not in _CACHED:
        _CACHED["nc"] = _build_kernel()
    return _CACHED["nc"]


def _make_in_maps(inputs) -> list:
    x = np.asarray(inputs["neighbr_feats"], dtype=np.float32)
    w = np.asarray(inputs["W"], dtype=np.float32)
    b = np.asarray(inputs["b"], dtype=np.float32)

    wt = np.ascontiguousarray(w.T.astype(np.float16))
    bias = np.ascontiguousarray(b.reshape(C_OUT, 1))
    maskL, maskR = _host_consts()

    in_maps = []
    for core in range(N_CORES):
        xs = np.ascontiguousarray(x[core * B_LOC:(core + 1) * B_LOC])
        in_maps.append({
            "X": xs, "WT": wt, "BIAS": bias, "ML": maskL, "MR": maskR,
        })
    return in_maps


def kernel(**inputs) -> np.ndarray:
    nc = _get_nc()
    in_maps = _make_in_maps(inputs)
    res = run_bass_kernel_spmd(nc, in_maps, core_ids=list(range(N_CORES)))
    out = np.concatenate([r["OUT"] for r in res.results], axis=0)
    return out.astype(np.float32)


if __name__ == "__main__":
    rng = np.random.default_rng(0)
    inputs = {
        "neighbr_feats": rng.standard_normal((B, C_IN, H, WW)).astype(np.float32),
        "W": (rng.standard_normal((C_OUT, C_IN)) * 0.05).astype(np.float32),
        "b": (rng.standard_normal((C_OUT,)) * 0.05).astype(np.float32),
    }
    out = kernel(**inputs)
    print("kernel ran:", out.shape, out.dtype)
